# revision 5
# baseline (speedup 1.0000x reference)
"""Gaussian falloff vortex-velocity kernel for Trainium2 (Bass/Tile).

Math per batch element b (single vortex y,x,tau,sig per batch):
    d1 = py - y;  d2 = px - x;  q = d1^2 + d2^2
    s  = tau * exp(-q/sig^2) / sqrt(q)
    out[..., 0] = s * d2;  out[..., 1] = -s * d1

On-chip formulation (per core: 8 batches, each [512,512,2] -> [128, 4096]):
    De  = y - py                      (ACT Identity: scale=-1, bias=y)     = -d1
    Do  = px - x                      (DVE tensor_scalar_sub)              =  d2
    Qe  = Square(De * (1/sig))        (ACT Square with AP scale)           = d1^2/sig^2
    Qo  = Square(Do * (1/sig))
    q'  = Qe + Qo                     (DVE tensor_tensor add)              = q/sig^2
    L   = Ln(q')                      (ACT Ln)
    z   = 0.5*L + q'                  (DVE scalar_tensor_tensor)
    s'  = Exp(-z + ln(tau/sig))       (ACT Exp, imm scale=-1, AP bias)
        = tau/sig * exp(-q') / sqrt(q') = tau * exp(-q/sig^2) / sqrt(q)
    out_even = s' * Do;  out_odd = s' * De   (DVE tensor_tensor, strided writes)

All ACT functions (identity, square, ln, exp) live in the single
`natural_log_exp_and_others` table set -> one table load.
"""

import numpy as np

import concourse.bass as bass
import concourse.bacc as bacc
import concourse.mybir as mybir
from concourse.tile import TileContext
from concourse.bass_utils import run_bass_kernel_spmd

N_CORES = 8
B_PER_CORE = 8          # 64 batches / 8 cores
P = 128                 # SBUF partitions
FD = 4096               # floats per partition for one batch ([512*512*2] / 128)
PTS = FD // 2           # points per partition
NCONST = 4              # y, x, 1/sig, ln(tau/sig)

_PROGRAM = None


def _build_program():
    f32 = mybir.dt.float32
    AF = mybir.ActivationFunctionType
    OP = mybir.AluOpType

    nc = bacc.Bacc(
        "TRN2",
        target_bir_lowering=False,
        debug=False,
        num_devices=N_CORES,
    )
    pts = nc.declare_dram_parameter("points", [B_PER_CORE * P, FD], f32, isOutput=False)
    cst = nc.declare_dram_parameter("consts", [P, NCONST * B_PER_CORE], f32, isOutput=False)
    out = nc.declare_dram_parameter("out", [B_PER_CORE * P, FD], f32, isOutput=True)

    with TileContext(nc) as tc:
        with (
            tc.tile_pool(name="cpool", bufs=1) as cpool,
            tc.tile_pool(name="io", bufs=2) as io,
            tc.tile_pool(name="tmp", bufs=2) as tmp,
        ):
            # Warm-up activation with no dependencies: walrus inserts the ACT
            # table load (natural_log_exp_and_others) before the first
            # activation; doing it here keeps the load off the critical path
            # and away from wait-heavy instructions (HW wait-slot limit).
            w = cpool.tile([P, 1], f32)
            nc.vector.memset(w[:], 1.0)
            nc.scalar.activation(w[:], w[:], AF.Exp)

            c = cpool.tile([P, NCONST * B_PER_CORE], f32)
            nc.sync.dma_start(c[:], cst[:])

            for b in range(B_PER_CORE):
                j = NCONST * b
                y_ap = c[:, j + 0 : j + 1]
                x_ap = c[:, j + 1 : j + 2]
                inv_ap = c[:, j + 2 : j + 3]
                lnts_ap = c[:, j + 3 : j + 4]

                T = io.tile([P, FD], f32, tag="T")
                nc.sync.dma_start(T[:], pts[b * P : (b + 1) * P, :])
                Tv = T.rearrange("p (n c) -> p n c", c=2)

                De = tmp.tile([P, PTS], f32, tag="De")
                Do = tmp.tile([P, PTS], f32, tag="Do")
                Qe = tmp.tile([P, PTS], f32, tag="Qe")
                Qo = tmp.tile([P, PTS], f32, tag="Qo")

                # De = y - py ; Do = px - x
                nc.scalar.activation(De[:], Tv[:, :, 0], AF.Identity, bias=y_ap, scale=-1.0)
                nc.vector.tensor_scalar_sub(Do[:], Tv[:, :, 1], x_ap)
                # q' = (d1/sig)^2 + (d2/sig)^2
                nc.scalar.activation(Qe[:], De[:], AF.Square, scale=inv_ap)
                nc.scalar.activation(Qo[:], Do[:], AF.Square, scale=inv_ap)
                nc.vector.tensor_tensor(Qe[:], Qe[:], Qo[:], OP.add)        # q' -> Qe
                nc.scalar.activation(Qo[:], Qe[:], AF.Ln)                   # L  -> Qo
                nc.vector.scalar_tensor_tensor(Qo[:], Qo[:], 0.5, Qe[:], OP.mult, OP.add)  # z -> Qo
                nc.scalar.activation(Qe[:], Qo[:], AF.Exp, bias=lnts_ap, scale=-1.0)       # s' -> Qe

                O = io.tile([P, FD], f32, tag="O")
                Ov = O.rearrange("p (n c) -> p n c", c=2)
                nc.vector.tensor_tensor(Ov[:, :, 0], Qe[:], Do[:], OP.mult)
                nc.vector.tensor_tensor(Ov[:, :, 1], Qe[:], De[:], OP.mult)
                nc.sync.dma_start(out[b * P : (b + 1) * P, :], O[:])

    nc.compile()
    return nc


def _get_program():
    global _PROGRAM
    if _PROGRAM is None:
        _PROGRAM = _build_program()
    return _PROGRAM


def _make_in_maps(vortex_feature, points):
    B, H, W, _ = points.shape
    vf = np.asarray(vortex_feature, dtype=np.float64).reshape(B, 6)
    y, x, tau, sig = vf[:, 0], vf[:, 1], vf[:, 2], vf[:, 3]
    sig_c = np.maximum(sig, 1e-35)  # sig==0 -> falloff 0; keep ln(tau/sig) finite
    inv = 1.0 / sig_c
    with np.errstate(divide="ignore"):
        lnts = np.log(tau) - np.log(sig_c)  # ln(tau/sig); tau==0 -> -inf (s'=0, correct)
    consts = np.stack([y, x, inv, lnts], axis=1).astype(np.float32)  # [B, 4]

    in_maps = []
    for i in range(N_CORES):
        sl = slice(i * B_PER_CORE, (i + 1) * B_PER_CORE)
        pshard = np.ascontiguousarray(points[sl]).reshape(B_PER_CORE * P, FD)
        cshard = np.ascontiguousarray(
            np.broadcast_to(consts[sl].reshape(1, NCONST * B_PER_CORE), (P, NCONST * B_PER_CORE))
        )
        in_maps.append({"points": pshard, "consts": cshard})
    return in_maps


def run(vortex_feature, points, trace=False):
    nc = _get_program()
    in_maps = _make_in_maps(vortex_feature, points)
    res = run_bass_kernel_spmd(nc, in_maps, list(range(N_CORES)), trace=trace)
    B, H, W, _ = points.shape
    out = np.empty((B, H, W, 2), dtype=np.float32)
    for i in range(N_CORES):
        sl = slice(i * B_PER_CORE, (i + 1) * B_PER_CORE)
        out[sl] = res.results[i]["out"].reshape(B_PER_CORE, H, W, 2)
    return out, res


def kernel(vortex_feature: np.ndarray, points: np.ndarray) -> np.ndarray:
    out, _ = run(vortex_feature, points, trace=False)
    return out
